# revision 17
# baseline (speedup 1.0000x reference)
"""
Multi-head attention (b=4, n=1024, e=768, h=12, dh=64) on 8 trn2 NeuronCores.

Sharding: (batch, head-group) -> core.  Core c handles batch b=c//2 and head
group g=c%2 (6 of the 12 heads).  Each core computes QKV projection for its
heads, attention, and a row-parallel slice of the output projection, producing
a partial [1024, 768] output.  The host sums the two partials per batch
(the row-parallel all-reduce) during unsharding.

All biases ride an augmented contraction row: inputs are padded to 896 (=7*128)
contraction rows where row 768 of xT is ones and row 768 of each weight holds
the bias (rows 769+ are zero).  The softmax denominator comes from an extra
ones-column appended to each head's V (column 64 of each 128-wide head slot),
so the column-sum of exp(E) falls out of the same PE matmul that computes A@V.
The softmax 1/sqrt(768) scale is folded into Wproj on the host.

Matmuls run as float32r (TF32 fast path on the PE).  Attention matmuls are
zero-padded to full 128x128 stationary shapes: half-array matmuls (K=64 E,
M=65 AV) keep the PE's HAM activity monitor below its warm threshold and the
whole attention phase gets clocked at 1.2 GHz instead of 2.4.
"""

import math
from contextlib import ExitStack

import numpy as np

import concourse.bass as bass  # noqa: F401
import concourse.mybir as mybir
import concourse.tile as tile
from concourse import bacc
from concourse.bass_utils import run_bass_kernel_spmd

EMB = 768
HEADS = 12
DH = 64
N = 1024
B = 4
HPC = 6  # heads per core
KC = 7  # contraction chunks (896 = 7*128) for the bias-augmented qkv matmuls
VW = HPC * 65 + 63  # V slots overlap: lhsT reads 128 cols from slot h*65
F32 = mybir.dt.float32
F32R = mybir.dt.float32r

N_CORES = 8


def build_program():
    nc = bacc.Bacc("TRN2", target_bir_lowering=False, debug=False, num_devices=N_CORES)

    xT = nc.dram_tensor("xT", [KC * 128, N], F32R, kind="ExternalInput").ap()
    Wq = nc.dram_tensor("Wq", [KC * 128, 384], F32R, kind="ExternalInput").ap()
    Wk = nc.dram_tensor("Wk", [KC * 128, 384], F32R, kind="ExternalInput").ap()
    Wv = nc.dram_tensor("Wv", [KC * 128, HPC * 65], F32R, kind="ExternalInput").ap()
    Wp = nc.dram_tensor("Wp", [4 * 128, EMB], F32R, kind="ExternalInput").ap()
    out = nc.dram_tensor("out", [N, EMB], F32, kind="ExternalOutput").ap()

    Exp = mybir.ActivationFunctionType.Exp

    with tile.TileContext(nc) as tc, ExitStack() as ctx:
        const = ctx.enter_context(tc.tile_pool(name="const", bufs=1))
        ldp = tc.alloc_tile_pool(name="ldp", bufs=1)

        # Phase-1-only SBUF tensors (pool released after the QKV projections)
        xT_sb = ldp.tile([128, KC, N], F32R)
        Wq_sb = ldp.tile([128, KC, 384], F32R)
        Wk_sb = ldp.tile([128, KC, 384], F32R)
        Wv_sb = ldp.tile([128, KC, HPC * 65], F32R)
        # Resident
        Wp_sb = const.tile([128, 4, EMB], F32R)
        # warmup operand: first DMA in flight (Wp rows 384.. are bias+zeros)
        warm_sb = const.tile([128, 640], F32R)
        nc.gpsimd.dma_start(warm_sb[:], Wp[384:512, 0:640])
        # per-chunk DMAs so QKV matmuls can start before the full load lands
        xT_r = xT.rearrange("(c p) n -> p c n", p=128)
        Wq_r = Wq.rearrange("(c p) n -> p c n", p=128)
        Wk_r = Wk.rearrange("(c p) n -> p c n", p=128)
        Wv_r = Wv.rearrange("(c p) n -> p c n", p=128)
        for c in range(KC):
            nc.sync.dma_start(xT_sb[:, c, :], xT_r[:, c, :])
            nc.sync.dma_start(Wq_sb[:, c, :], Wq_r[:, c, :])
            nc.sync.dma_start(Wk_sb[:, c, :], Wk_r[:, c, :])
        for c in range(KC):
            nc.sync.dma_start(Wv_sb[:, c, :], Wv_r[:, c, :])
        nc.sync.dma_start(Wp_sb[:], Wp.rearrange("(c p) n -> p c n", p=128))

        # Per-head padded Q^T/K^T: head h in partitions 0..63, zeros in 64..127
        # so attention matmuls present full 128x128 stationary shapes to the PE
        # (half-array shapes keep the HAM throttle engaged).
        QT_sb = const.tile([128, HPC, N], F32R)
        KT_sb = const.tile([128, HPC, N], F32R)
        V_sb = const.tile([128, 8, VW], F32R)  # V natural, 8 token chunks
        # tail pad so slot 5's 128-wide lhsT read is fully initialized (any
        # finite values work: it only feeds unread PSUM rows)
        for t8 in range(8):
            nc.gpsimd.dma_start(V_sb[:, t8, HPC * 65 :], Wp[384:512, 0:63])
        OT_sb = const.tile([128, 4, N], F32R)  # scaled O^T; chunk 3 = [ones; zeros]

        # Zero the padded partitions of QT/KT (both sides must be real zeros:
        # 0 * NaN-garbage would still poison the contraction).  Rows 769.. of
        # xT are zeros by construction, so DMA them in instead of burning DVE
        # time.  The V slot padding columns only feed PSUM rows 65..127, which
        # are never read, so they can stay uninitialized.
        for h in range(HPC):
            nc.gpsimd.dma_start(QT_sb[64:128, h, :], xT[769 : 769 + 64, :])
            nc.gpsimd.dma_start(KT_sb[64:128, h, :], xT[769 : 769 + 64, :])
        ones_scratch = const.tile([128, N], F32)
        nc.vector.memset(ones_scratch[:], 0.0)
        nc.vector.memset(ones_scratch[0:1, :], 1.0)
        nc.vector.tensor_copy(OT_sb[:, 3, :], ones_scratch[:])

        # ---- Phase 1: QKV projections -------------------------------------
        with tc.tile_pool(name="psum1", bufs=1, space="PSUM") as pp1:
            # warmup junk matmuls: the PE would otherwise idle for ~12us while
            # the input DMAs land, leaving the HAM clock-gate cold for the
            # first half of the QKV phase
            for w in range(20):
                pw = pp1.tile([128, 512], F32, tag="warm", bufs=1, name=f"pw_{w}")
                nc.tensor.matmul(
                    pw[:], warm_sb[:, 0:128], warm_sb[:, 128:640],
                    start=True, stop=True,
                )
            def emit_v_group(t8):
                psv = pp1.tile([128, HPC * 65], F32, tag="v", bufs=2, name=f"psv_{t8}")
                for c in range(KC):
                    nc.tensor.matmul(
                        psv[:],
                        xT_sb[:, c, t8 * 128 : (t8 + 1) * 128],
                        Wv_sb[:, c, :],
                        start=(c == 0),
                        stop=(c == KC - 1),
                    )
                nc.vector.tensor_copy(V_sb[:, t8, 0 : HPC * 65], psv[:])

            for p3 in range(3):  # head-pair M chunks
                for qc in range(2):
                    psq = pp1.tile([128, 512], F32, tag="qk", bufs=3)
                    for c in range(KC):
                        nc.tensor.matmul(
                            psq[:],
                            Wq_sb[:, c, p3 * 128 : (p3 + 1) * 128],
                            xT_sb[:, c, qc * 512 : (qc + 1) * 512],
                            start=(c == 0),
                            stop=(c == KC - 1),
                        )
                    for hh in range(2):
                        nc.vector.tensor_copy(
                            QT_sb[0:64, 2 * p3 + hh, qc * 512 : (qc + 1) * 512],
                            psq[64 * hh : 64 * hh + 64, :],
                        )
                    psk = pp1.tile([128, 512], F32, tag="qk", bufs=3)
                    for c in range(KC):
                        nc.tensor.matmul(
                            psk[:],
                            Wk_sb[:, c, p3 * 128 : (p3 + 1) * 128],
                            xT_sb[:, c, qc * 512 : (qc + 1) * 512],
                            start=(c == 0),
                            stop=(c == KC - 1),
                        )
                    for hh in range(2):
                        nc.vector.tensor_copy(
                            KT_sb[0:64, 2 * p3 + hh, qc * 512 : (qc + 1) * 512],
                            psk[64 * hh : 64 * hh + 64, :],
                        )
                if p3 == 0:  # V right after head pair 0 so attention can start
                    for t8 in range(8):
                        emit_v_group(t8)
        ldp.release()

        # ---- Phase 2: attention per head ----------------------------------
        expp = ctx.enter_context(tc.tile_pool(name="expp", bufs=3))
        rpool = ctx.enter_context(tc.tile_pool(name="rpool", bufs=1))
        with tc.tile_pool(name="psum2", bufs=1, space="PSUM") as pp2:

            def emit_e_chunk(h, kc, junk=False):
                """E^T block (k-chunk kc) for head h -> exp tile in SBUF.

                junk=True emits the matmuls but no exp and returns None: used
                at the end of the attention phase purely to keep the PE's HAM
                activity monitor warm through the proj-phase transition."""
                pe = pp2.tile([128, N], F32, tag="e", bufs=2, name=f"pe_{h}_{kc}")
                for qc in range(2):
                    nc.tensor.matmul(
                        pe[:, qc * 512 : (qc + 1) * 512],
                        KT_sb[:, h, kc * 128 : (kc + 1) * 128],
                        QT_sb[:, h, qc * 512 : (qc + 1) * 512],
                        start=True,
                        stop=True,
                    )
                if junk:
                    return None
                ex = expp.tile([128, N], F32R, tag="ex", name=f"ex_{h}_{kc}")
                nc.scalar.activation(ex[:], pe[:], Exp)
                return ex

            # software pipeline: E-matmuls run one chunk ahead of AV-matmuls so
            # the PE always has exp-independent work while ACT computes exp.
            ex_next = emit_e_chunk(0, 0)
            for h in range(HPC):
                p3, half = divmod(h, 2)
                off = 64 * half
                po = pp2.tile([128, N], F32, tag="o", bufs=2, name=f"po_{h}")
                for kc in range(8):
                    ex = ex_next
                    nh, nkc = (h, kc + 1) if kc < 7 else (h + 1, 0)
                    if nh < HPC:
                        ex_next = emit_e_chunk(nh, nkc)
                    else:
                        # keepalive: dense PE work through the pipeline drain
                        emit_e_chunk(HPC - 1, nkc if kc < 7 else 0, junk=True)
                    for qc in range(2):
                        nc.tensor.matmul(
                            po[:, qc * 512 : (qc + 1) * 512],
                            V_sb[:, kc, h * 65 : h * 65 + 128],
                            ex[:, qc * 512 : (qc + 1) * 512],
                            start=(kc == 0),
                            stop=(kc == 7),
                        )
                # softmax normalization: row 64 of po holds sum_k exp(E^T).
                # approx-fast reciprocal (~4e-6 rel err) is far below the TF32
                # matmul noise and ~5x faster than the exact DVE reciprocal.
                # It misreads nonzero partition offsets on HW, so copy the sums
                # row down to partition 0 first.
                ss = rpool.tile([1, N], F32, tag="ss")
                nc.vector.tensor_copy(ss[:], po[64:65, :])
                rs = rpool.tile([1, N], F32, tag="rs")
                nc.vector.reciprocal_approx_fast(rs[:], ss[:])
                rb = rpool.tile([64, N], F32, tag="rb")
                nc.gpsimd.partition_broadcast(rb[:], rs[:])
                nc.vector.tensor_mul(
                    OT_sb[off : off + 64, p3, :], po[0:64, :], rb[:]
                )
            # bridge the last head's normalization chain (PE would idle ~5us
            # before the proj phase's first matmul, re-engaging the throttle)
            for w in range(10):
                emit_e_chunk(HPC - 1, w % 8, junk=True)

        # ---- Phase 3: output projection -----------------------------------
        outp = ctx.enter_context(tc.tile_pool(name="outp", bufs=3))
        with tc.tile_pool(name="psum3", bufs=1, space="PSUM") as pp3:
            for qc8 in range(8):
                pso = pp3.tile([128, EMB], F32, tag="p", bufs=3)
                for n0, n1 in ((0, 512), (512, 768)):
                    for c in range(4):
                        nc.tensor.matmul(
                            pso[:, n0:n1],
                            OT_sb[:, c, qc8 * 128 : (qc8 + 1) * 128],
                            Wp_sb[:, c, n0:n1],
                            start=(c == 0),
                            stop=(c == 3),
                        )
                ot = outp.tile([128, EMB], F32, tag="out")
                if qc8 % 2 == 0:
                    nc.vector.tensor_copy(ot[:], pso[:])
                else:
                    nc.scalar.copy(ot[:], pso[:])
                nc.sync.dma_start(out[qc8 * 128 : (qc8 + 1) * 128, :], ot[:])

    nc.compile()
    return nc


def build_in_maps(x, Wqkv, bqkv, Wproj, bproj):
    x = np.asarray(x, dtype=np.float32)
    Wqkv = np.asarray(Wqkv, dtype=np.float32)
    bqkv = np.asarray(bqkv, dtype=np.float32)
    Wproj = np.asarray(Wproj, dtype=np.float32)
    bproj = np.asarray(bproj, dtype=np.float32)

    s = 1.0 / math.sqrt(EMB)
    cols = np.arange(3 * EMB).reshape(HEADS, DH, 3)  # (h, d, qkv) col index map
    in_maps = []
    for c in range(N_CORES):
        b, g = divmod(c, 2)
        hsl = slice(g * HPC, (g + 1) * HPC)
        qcols = cols[hsl, :, 0].reshape(-1)
        kcols = cols[hsl, :, 1].reshape(-1)
        vcols = cols[hsl, :, 2]  # [HPC, DH]

        xT_a = np.zeros((KC * 128, N), np.float32)
        xT_a[:EMB] = x[b].T
        xT_a[EMB] = 1.0

        Wq_a = np.zeros((KC * 128, 384), np.float32)
        Wq_a[:EMB] = Wqkv[:, qcols]
        Wq_a[EMB] = bqkv[qcols]
        Wk_a = np.zeros((KC * 128, 384), np.float32)
        Wk_a[:EMB] = Wqkv[:, kcols]
        Wk_a[EMB] = bqkv[kcols]

        Wv_a = np.zeros((KC * 128, HPC * 65), np.float32)
        for j in range(HPC):
            Wv_a[:EMB, j * 65 : j * 65 + DH] = Wqkv[:, vcols[j]]
            Wv_a[EMB, j * 65 : j * 65 + DH] = bqkv[vcols[j]]
            Wv_a[EMB, j * 65 + DH] = 1.0

        Wp_a = np.zeros((4 * 128, EMB), np.float32)
        Wp_a[:384] = Wproj[g * 384 : (g + 1) * 384] * s
        if g == 0:
            Wp_a[384] = bproj

        in_maps.append({"xT": xT_a, "Wq": Wq_a, "Wk": Wk_a, "Wv": Wv_a, "Wp": Wp_a})
    return in_maps


_NC_CACHE = None


def _get_program():
    global _NC_CACHE
    if _NC_CACHE is None:
        _NC_CACHE = build_program()
    return _NC_CACHE


def kernel(x, Wqkv, bqkv, Wproj, bproj, **_kwargs):
    nc = _get_program()
    in_maps = build_in_maps(x, Wqkv, bqkv, Wproj, bproj)
    res = run_bass_kernel_spmd(nc, in_maps, list(range(N_CORES))).results
    out = np.empty((B, N, EMB), np.float32)
    for b in range(B):
        out[b] = res[2 * b]["out"] + res[2 * b + 1]["out"]
    return out


# revision 18
# speedup vs baseline: 1.1166x; 1.1166x over previous
"""
Multi-head attention (b=4, n=1024, e=768, h=12, dh=64) on 8 trn2 NeuronCores.

Sharding: (batch, head-group) -> core.  Core c handles batch b=c//2 and head
group g=c%2 (6 of the 12 heads).  Each core computes QKV projection for its
heads, attention, and a row-parallel slice of the output projection, producing
a partial [1024, 768] output.  The host sums the two partials per batch
(the row-parallel all-reduce) during unsharding.

All biases ride an augmented contraction row: inputs are padded to 896 (=7*128)
contraction rows where row 768 of xT is ones and row 768 of each weight holds
the bias (rows 769+ are zero).  The softmax denominator comes from an extra
ones-column appended to each head's V (column 64 of each 128-wide head slot),
so the column-sum of exp(E) falls out of the same PE matmul that computes A@V.
The softmax 1/sqrt(768) scale is folded into Wproj on the host.

Matmuls run as float32r (TF32 fast path on the PE).  Attention matmuls are
zero-padded to full 128x128 stationary shapes: half-array matmuls (K=64 E,
M=65 AV) keep the PE's HAM activity monitor below its warm threshold and the
whole attention phase gets clocked at 1.2 GHz instead of 2.4.
"""

import math
from contextlib import ExitStack

import numpy as np

import concourse.bass as bass  # noqa: F401
import concourse.mybir as mybir
import concourse.tile as tile
from concourse import bacc
from concourse.bass_utils import run_bass_kernel_spmd

EMB = 768
HEADS = 12
DH = 64
N = 1024
B = 4
HPC = 6  # heads per core
KC = 7  # contraction chunks (896 = 7*128) for the bias-augmented qkv matmuls
VW = HPC * 65 + 63  # V slots overlap: lhsT reads 128 cols from slot h*65
F32 = mybir.dt.float32
F32R = mybir.dt.float32r

N_CORES = 8


def build_program():
    nc = bacc.Bacc("TRN2", target_bir_lowering=False, debug=False, num_devices=N_CORES)

    xT = nc.dram_tensor("xT", [KC * 128, N], F32R, kind="ExternalInput").ap()
    Wq = nc.dram_tensor("Wq", [KC * 128, 384], F32R, kind="ExternalInput").ap()
    Wk = nc.dram_tensor("Wk", [KC * 128, 384], F32R, kind="ExternalInput").ap()
    Wv = nc.dram_tensor("Wv", [KC * 128, HPC * 65], F32R, kind="ExternalInput").ap()
    Wp = nc.dram_tensor("Wp", [4 * 128, EMB], F32R, kind="ExternalInput").ap()
    out = nc.dram_tensor("out", [N, EMB], F32, kind="ExternalOutput").ap()

    Exp = mybir.ActivationFunctionType.Exp

    with tile.TileContext(nc) as tc, ExitStack() as ctx:
        const = ctx.enter_context(tc.tile_pool(name="const", bufs=1))
        ldp = tc.alloc_tile_pool(name="ldp", bufs=1)

        # Phase-1-only SBUF tensors (pool released after the QKV projections)
        xT_sb = ldp.tile([128, KC, N], F32R)
        Wq_sb = ldp.tile([128, KC, 384], F32R)
        Wk_sb = ldp.tile([128, KC, 384], F32R)
        Wv_sb = ldp.tile([128, KC, HPC * 65], F32R)
        # Resident
        Wp_sb = const.tile([128, 4, EMB], F32R)
        # warmup operand: first DMA in flight (Wp rows 384.. are bias+zeros)
        warm_sb = const.tile([128, 640], F32R)
        nc.sync.dma_start(warm_sb[:], Wp[384:512, 0:640])
        # per-chunk DMAs so QKV matmuls can start before the full load lands
        xT_r = xT.rearrange("(c p) n -> p c n", p=128)
        Wq_r = Wq.rearrange("(c p) n -> p c n", p=128)
        Wk_r = Wk.rearrange("(c p) n -> p c n", p=128)
        Wv_r = Wv.rearrange("(c p) n -> p c n", p=128)
        for c in range(KC):
            nc.sync.dma_start(xT_sb[:, c, :], xT_r[:, c, :])
            nc.sync.dma_start(Wq_sb[:, c, :], Wq_r[:, c, :])
            nc.sync.dma_start(Wk_sb[:, c, :], Wk_r[:, c, :])
        for c in range(KC):
            nc.sync.dma_start(Wv_sb[:, c, :], Wv_r[:, c, :])
        nc.sync.dma_start(Wp_sb[:], Wp.rearrange("(c p) n -> p c n", p=128))

        # Per-head padded Q^T/K^T: head h in partitions 0..63, zeros in 64..127
        # so attention matmuls present full 128x128 stationary shapes to the PE
        # (half-array shapes keep the HAM throttle engaged).
        QT_sb = const.tile([128, HPC, N], F32R)
        KT_sb = const.tile([128, HPC, N], F32R)
        V_sb = const.tile([128, 8, VW], F32R)  # V natural, 8 token chunks
        OT_pad_marker = None  # (V tails + QT/KT pads via DVE below)
        OT_sb = const.tile([128, 4, N], F32R)  # scaled O^T; chunk 3 = [ones; zeros]

        # Zero the padded partitions of QT/KT (both sides must be real zeros:
        # 0 * NaN-garbage would still poison the contraction).  Rows 769.. of
        # xT are zeros by construction, so DMA them in instead of burning DVE
        # time.  The V slot padding columns only feed PSUM rows 65..127, which
        # are never read, so they can stay uninitialized.
        ones_scratch = const.tile([128, N], F32)
        nc.vector.memset(ones_scratch[:], 0.0)
        nc.vector.memset(ones_scratch[0:1, :], 1.0)
        nc.vector.tensor_copy(OT_sb[:, 3, :], ones_scratch[:])
        for h in range(HPC):
            nc.vector.tensor_copy(QT_sb[64:128, h, :], ones_scratch[64:128, :])
            nc.vector.tensor_copy(KT_sb[64:128, h, :], ones_scratch[64:128, :])
        for t8 in range(8):
            nc.vector.tensor_copy(V_sb[:, t8, HPC * 65 :], ones_scratch[:, 0:63])

        # ---- Phase 1: QKV projections -------------------------------------
        with tc.tile_pool(name="psum1", bufs=1, space="PSUM") as pp1:
            # warmup junk matmuls: the PE would otherwise idle for ~12us while
            # the input DMAs land, leaving the HAM clock-gate cold for the
            # first half of the QKV phase
            for w in range(20):
                pw = pp1.tile([128, 512], F32, tag="warm", bufs=1, name=f"pw_{w}")
                nc.tensor.matmul(
                    pw[:], warm_sb[:, 0:128], warm_sb[:, 128:640],
                    start=True, stop=True,
                )
            def emit_v_group(t8):
                psv = pp1.tile([128, HPC * 65], F32, tag="v", bufs=2, name=f"psv_{t8}")
                for c in range(KC):
                    nc.tensor.matmul(
                        psv[:],
                        xT_sb[:, c, t8 * 128 : (t8 + 1) * 128],
                        Wv_sb[:, c, :],
                        start=(c == 0),
                        stop=(c == KC - 1),
                    )
                nc.vector.tensor_copy(V_sb[:, t8, 0 : HPC * 65], psv[:])

            for p3 in range(3):  # head-pair M chunks
                for qc in range(2):
                    psq = pp1.tile([128, 512], F32, tag="qk", bufs=3)
                    for c in range(KC):
                        nc.tensor.matmul(
                            psq[:],
                            Wq_sb[:, c, p3 * 128 : (p3 + 1) * 128],
                            xT_sb[:, c, qc * 512 : (qc + 1) * 512],
                            start=(c == 0),
                            stop=(c == KC - 1),
                        )
                    for hh in range(2):
                        nc.vector.tensor_copy(
                            QT_sb[0:64, 2 * p3 + hh, qc * 512 : (qc + 1) * 512],
                            psq[64 * hh : 64 * hh + 64, :],
                        )
                    psk = pp1.tile([128, 512], F32, tag="qk", bufs=3)
                    for c in range(KC):
                        nc.tensor.matmul(
                            psk[:],
                            Wk_sb[:, c, p3 * 128 : (p3 + 1) * 128],
                            xT_sb[:, c, qc * 512 : (qc + 1) * 512],
                            start=(c == 0),
                            stop=(c == KC - 1),
                        )
                    for hh in range(2):
                        nc.vector.tensor_copy(
                            KT_sb[0:64, 2 * p3 + hh, qc * 512 : (qc + 1) * 512],
                            psk[64 * hh : 64 * hh + 64, :],
                        )
                if p3 == 0:  # V right after head pair 0 so attention can start
                    for t8 in range(8):
                        emit_v_group(t8)
        ldp.release()

        # ---- Phase 2: attention per head ----------------------------------
        expp = ctx.enter_context(tc.tile_pool(name="expp", bufs=3))
        rpool = ctx.enter_context(tc.tile_pool(name="rpool", bufs=1))
        with tc.tile_pool(name="psum2", bufs=1, space="PSUM") as pp2:

            def emit_e_chunk(h, kc, junk=False):
                """E^T block (k-chunk kc) for head h -> exp tile in SBUF.

                junk=True emits the matmuls but no exp and returns None: used
                at the end of the attention phase purely to keep the PE's HAM
                activity monitor warm through the proj-phase transition."""
                pe = pp2.tile([128, N], F32, tag="e", bufs=2, name=f"pe_{h}_{kc}")
                for qc in range(2):
                    nc.tensor.matmul(
                        pe[:, qc * 512 : (qc + 1) * 512],
                        KT_sb[:, h, kc * 128 : (kc + 1) * 128],
                        QT_sb[:, h, qc * 512 : (qc + 1) * 512],
                        start=True,
                        stop=True,
                    )
                if junk:
                    return None
                ex = expp.tile([128, N], F32R, tag="ex", name=f"ex_{h}_{kc}")
                nc.scalar.activation(ex[:], pe[:], Exp)
                return ex

            # software pipeline: E-matmuls run one chunk ahead of AV-matmuls so
            # the PE always has exp-independent work while ACT computes exp.
            ex_next = emit_e_chunk(0, 0)
            for h in range(HPC):
                p3, half = divmod(h, 2)
                off = 64 * half
                po = pp2.tile([128, N], F32, tag="o", bufs=2, name=f"po_{h}")
                for kc in range(8):
                    ex = ex_next
                    nh, nkc = (h, kc + 1) if kc < 7 else (h + 1, 0)
                    if nh < HPC:
                        ex_next = emit_e_chunk(nh, nkc)
                    else:
                        # keepalive: dense PE work through the pipeline drain
                        emit_e_chunk(HPC - 1, nkc if kc < 7 else 0, junk=True)
                    for qc in range(2):
                        nc.tensor.matmul(
                            po[:, qc * 512 : (qc + 1) * 512],
                            V_sb[:, kc, h * 65 : h * 65 + 128],
                            ex[:, qc * 512 : (qc + 1) * 512],
                            start=(kc == 0),
                            stop=(kc == 7),
                        )
                # softmax normalization: row 64 of po holds sum_k exp(E^T).
                # approx-fast reciprocal (~4e-6 rel err) is far below the TF32
                # matmul noise and ~5x faster than the exact DVE reciprocal.
                # It misreads nonzero partition offsets on HW, so copy the sums
                # row down to partition 0 first.
                ss = rpool.tile([1, N], F32, tag="ss")
                nc.vector.tensor_copy(ss[:], po[64:65, :])
                rs = rpool.tile([1, N], F32, tag="rs")
                nc.vector.reciprocal_approx_fast(rs[:], ss[:])
                rb = rpool.tile([64, N], F32, tag="rb")
                nc.gpsimd.partition_broadcast(rb[:], rs[:])
                nc.vector.tensor_mul(
                    OT_sb[off : off + 64, p3, :], po[0:64, :], rb[:]
                )
            # bridge the last head's normalization chain (PE would idle ~5us
            # before the proj phase's first matmul, re-engaging the throttle)
            for w in range(10):
                emit_e_chunk(HPC - 1, w % 8, junk=True)

        # ---- Phase 3: output projection -----------------------------------
        outp = ctx.enter_context(tc.tile_pool(name="outp", bufs=3))
        with tc.tile_pool(name="psum3", bufs=1, space="PSUM") as pp3:
            for qc8 in range(8):
                pso = pp3.tile([128, EMB], F32, tag="p", bufs=3)
                for n0, n1 in ((0, 512), (512, 768)):
                    for c in range(4):
                        nc.tensor.matmul(
                            pso[:, n0:n1],
                            OT_sb[:, c, qc8 * 128 : (qc8 + 1) * 128],
                            Wp_sb[:, c, n0:n1],
                            start=(c == 0),
                            stop=(c == 3),
                        )
                ot = outp.tile([128, EMB], F32, tag="out")
                if qc8 % 2 == 0:
                    nc.vector.tensor_copy(ot[:], pso[:])
                else:
                    nc.scalar.copy(ot[:], pso[:])
                nc.sync.dma_start(out[qc8 * 128 : (qc8 + 1) * 128, :], ot[:])

    nc.compile()
    return nc


def build_in_maps(x, Wqkv, bqkv, Wproj, bproj):
    x = np.asarray(x, dtype=np.float32)
    Wqkv = np.asarray(Wqkv, dtype=np.float32)
    bqkv = np.asarray(bqkv, dtype=np.float32)
    Wproj = np.asarray(Wproj, dtype=np.float32)
    bproj = np.asarray(bproj, dtype=np.float32)

    s = 1.0 / math.sqrt(EMB)
    cols = np.arange(3 * EMB).reshape(HEADS, DH, 3)  # (h, d, qkv) col index map
    in_maps = []
    for c in range(N_CORES):
        b, g = divmod(c, 2)
        hsl = slice(g * HPC, (g + 1) * HPC)
        qcols = cols[hsl, :, 0].reshape(-1)
        kcols = cols[hsl, :, 1].reshape(-1)
        vcols = cols[hsl, :, 2]  # [HPC, DH]

        xT_a = np.zeros((KC * 128, N), np.float32)
        xT_a[:EMB] = x[b].T
        xT_a[EMB] = 1.0

        Wq_a = np.zeros((KC * 128, 384), np.float32)
        Wq_a[:EMB] = Wqkv[:, qcols]
        Wq_a[EMB] = bqkv[qcols]
        Wk_a = np.zeros((KC * 128, 384), np.float32)
        Wk_a[:EMB] = Wqkv[:, kcols]
        Wk_a[EMB] = bqkv[kcols]

        Wv_a = np.zeros((KC * 128, HPC * 65), np.float32)
        for j in range(HPC):
            Wv_a[:EMB, j * 65 : j * 65 + DH] = Wqkv[:, vcols[j]]
            Wv_a[EMB, j * 65 : j * 65 + DH] = bqkv[vcols[j]]
            Wv_a[EMB, j * 65 + DH] = 1.0

        Wp_a = np.zeros((4 * 128, EMB), np.float32)
        Wp_a[:384] = Wproj[g * 384 : (g + 1) * 384] * s
        if g == 0:
            Wp_a[384] = bproj

        in_maps.append({"xT": xT_a, "Wq": Wq_a, "Wk": Wk_a, "Wv": Wv_a, "Wp": Wp_a})
    return in_maps


_NC_CACHE = None


def _get_program():
    global _NC_CACHE
    if _NC_CACHE is None:
        _NC_CACHE = build_program()
    return _NC_CACHE


def kernel(x, Wqkv, bqkv, Wproj, bproj, **_kwargs):
    nc = _get_program()
    in_maps = build_in_maps(x, Wqkv, bqkv, Wproj, bproj)
    res = run_bass_kernel_spmd(nc, in_maps, list(range(N_CORES))).results
    out = np.empty((B, N, EMB), np.float32)
    for b in range(B):
        out[b] = res[2 * b]["out"] + res[2 * b + 1]["out"]
    return out


# revision 20
# speedup vs baseline: 1.1534x; 1.0329x over previous
"""
Multi-head attention (b=4, n=1024, e=768, h=12, dh=64) on 8 trn2 NeuronCores.

Sharding: (batch, head-group) -> core.  Core c handles batch b=c//2 and head
group g=c%2 (6 of the 12 heads).  Each core computes QKV projection for its
heads, attention, and a row-parallel slice of the output projection, producing
a partial [1024, 768] output.  The host sums the two partials per batch
(the row-parallel all-reduce) during unsharding.

All biases ride an augmented contraction row: inputs are padded to 896 (=7*128)
contraction rows where row 768 of xT is ones and row 768 of each weight holds
the bias (rows 769+ are zero).  The softmax denominator comes from an extra
ones-column appended to each head's V (column 64 of each 128-wide head slot),
so the column-sum of exp(E) falls out of the same PE matmul that computes A@V.
The softmax 1/sqrt(768) scale is folded into Wproj on the host.

Matmuls run as float32r (TF32 fast path on the PE).  Attention matmuls are
zero-padded to full 128x128 stationary shapes: half-array matmuls (K=64 E,
M=65 AV) keep the PE's HAM activity monitor below its warm threshold and the
whole attention phase gets clocked at 1.2 GHz instead of 2.4.
"""

import math
from contextlib import ExitStack

import numpy as np

import concourse.bass as bass  # noqa: F401
import concourse.mybir as mybir
import concourse.tile as tile
from concourse import bacc
from concourse.bass_utils import run_bass_kernel_spmd

EMB = 768
HEADS = 12
DH = 64
N = 1024
B = 4
HPC = 6  # heads per core
KC = 7  # contraction chunks (896 = 7*128) for the bias-augmented qkv matmuls
VW = HPC * 65 + 63  # V slots overlap: lhsT reads 128 cols from slot h*65
F32 = mybir.dt.float32
F32R = mybir.dt.float32r

N_CORES = 8


def build_program():
    nc = bacc.Bacc("TRN2", target_bir_lowering=False, debug=False, num_devices=N_CORES)

    xT = nc.dram_tensor("xT", [KC * 128, N], F32R, kind="ExternalInput").ap()
    Wq = nc.dram_tensor("Wq", [KC * 128, 384], F32R, kind="ExternalInput").ap()
    Wk = nc.dram_tensor("Wk", [KC * 128, 384], F32R, kind="ExternalInput").ap()
    Wv = nc.dram_tensor("Wv", [KC * 128, HPC * 65], F32R, kind="ExternalInput").ap()
    Wp = nc.dram_tensor("Wp", [4 * 128, EMB], F32R, kind="ExternalInput").ap()
    out = nc.dram_tensor("out", [N, EMB], F32, kind="ExternalOutput").ap()

    Exp = mybir.ActivationFunctionType.Exp

    with tile.TileContext(nc) as tc, ExitStack() as ctx:
        const = ctx.enter_context(tc.tile_pool(name="const", bufs=1))
        ldp = tc.alloc_tile_pool(name="ldp", bufs=1)

        # Phase-1-only SBUF tensors (pool released after the QKV projections)
        xT_sb = ldp.tile([128, KC, N], F32R)
        Wq_sb = ldp.tile([128, KC, 384], F32R)
        Wk_sb = ldp.tile([128, KC, 384], F32R)
        Wv_sb = ldp.tile([128, KC, HPC * 65], F32R)
        # Resident
        Wp_sb = const.tile([128, 4, EMB], F32R)
        # warmup operand: first DMA in flight (Wp rows 384.. are bias+zeros)
        warm_sb = const.tile([128, 640], F32R)
        nc.sync.dma_start(warm_sb[:], Wp[384:512, 0:640])
        # per-chunk DMAs so QKV matmuls can start before the full load lands
        xT_r = xT.rearrange("(c p) n -> p c n", p=128)
        Wq_r = Wq.rearrange("(c p) n -> p c n", p=128)
        Wk_r = Wk.rearrange("(c p) n -> p c n", p=128)
        Wv_r = Wv.rearrange("(c p) n -> p c n", p=128)
        for c in range(KC):
            nc.sync.dma_start(xT_sb[:, c, :], xT_r[:, c, :])
            nc.sync.dma_start(Wq_sb[:, c, :], Wq_r[:, c, :])
            nc.sync.dma_start(Wk_sb[:, c, :], Wk_r[:, c, :])
        for c in range(KC):
            nc.sync.dma_start(Wv_sb[:, c, :], Wv_r[:, c, :])
        nc.sync.dma_start(Wp_sb[:], Wp.rearrange("(c p) n -> p c n", p=128))

        # Per-head padded Q^T/K^T: head h in partitions 0..63, zeros in 64..127
        # so attention matmuls present full 128x128 stationary shapes to the PE
        # (half-array shapes keep the HAM throttle engaged).
        QT_sb = const.tile([128, HPC, N], F32R)
        KT_sb = const.tile([128, HPC, N], F32R)
        V_sb = const.tile([128, 8, VW], F32R)  # V natural, 8 token chunks
        OT_pad_marker = None  # (V tails + QT/KT pads via DVE below)
        OT_sb = const.tile([128, 4, N], F32R)  # scaled O^T; chunk 3 = [ones; zeros]

        # Zero the padded partitions of QT/KT (both sides must be real zeros:
        # 0 * NaN-garbage would still poison the contraction).  Rows 769.. of
        # xT are zeros by construction, so DMA them in instead of burning DVE
        # time.  The V slot padding columns only feed PSUM rows 65..127, which
        # are never read, so they can stay uninitialized.
        ones_scratch = const.tile([128, N], F32)
        nc.vector.memset(ones_scratch[:], 0.0)
        nc.vector.memset(ones_scratch[0:1, :], 1.0)
        nc.vector.tensor_copy(OT_sb[:, 3, :], ones_scratch[:])
        for h in range(HPC):
            nc.vector.tensor_copy(QT_sb[64:128, h, :], ones_scratch[64:128, :])
            nc.vector.tensor_copy(KT_sb[64:128, h, :], ones_scratch[64:128, :])
        for t8 in range(8):
            nc.vector.tensor_copy(V_sb[:, t8, HPC * 65 :], ones_scratch[:, 0:63])

        # ---- Phase 1: QKV projections -------------------------------------
        with tc.tile_pool(name="psum1", bufs=1, space="PSUM") as pp1:
            # warmup junk matmuls: the PE would otherwise idle for ~12us while
            # the input DMAs land, leaving the HAM clock-gate cold for the
            # first half of the QKV phase
            for w in range(20):
                pw = pp1.tile([128, 512], F32, tag="warm", bufs=1, name=f"pw_{w}")
                nc.tensor.matmul(
                    pw[:], warm_sb[:, 0:128], warm_sb[:, 128:640],
                    start=True, stop=True,
                )
            def emit_v_group(t8):
                psv = pp1.tile([128, HPC * 65], F32, tag="v", bufs=2, name=f"psv_{t8}")
                for c in range(KC):
                    nc.tensor.matmul(
                        psv[:],
                        xT_sb[:, c, t8 * 128 : (t8 + 1) * 128],
                        Wv_sb[:, c, :],
                        start=(c == 0),
                        stop=(c == KC - 1),
                    )
                nc.vector.tensor_copy(V_sb[:, t8, 0 : HPC * 65], psv[:])

            for p3 in range(3):  # head-pair M chunks
                for qc in range(2):
                    psq = pp1.tile([128, 512], F32, tag="qk", bufs=3)
                    for c in range(KC):
                        nc.tensor.matmul(
                            psq[:],
                            Wq_sb[:, c, p3 * 128 : (p3 + 1) * 128],
                            xT_sb[:, c, qc * 512 : (qc + 1) * 512],
                            start=(c == 0),
                            stop=(c == KC - 1),
                        )
                    nc.vector.tensor_copy(
                        QT_sb[0:64, 2 * p3, qc * 512 : (qc + 1) * 512],
                        psq[0:64, :],
                    )
                    nc.scalar.copy(
                        QT_sb[0:64, 2 * p3 + 1, qc * 512 : (qc + 1) * 512],
                        psq[64:128, :],
                    )
                    psk = pp1.tile([128, 512], F32, tag="qk", bufs=3)
                    for c in range(KC):
                        nc.tensor.matmul(
                            psk[:],
                            Wk_sb[:, c, p3 * 128 : (p3 + 1) * 128],
                            xT_sb[:, c, qc * 512 : (qc + 1) * 512],
                            start=(c == 0),
                            stop=(c == KC - 1),
                        )
                    nc.vector.tensor_copy(
                        KT_sb[0:64, 2 * p3, qc * 512 : (qc + 1) * 512],
                        psk[0:64, :],
                    )
                    nc.scalar.copy(
                        KT_sb[0:64, 2 * p3 + 1, qc * 512 : (qc + 1) * 512],
                        psk[64:128, :],
                    )
                if p3 == 0:  # V right after head pair 0 so attention can start
                    for t8 in range(8):
                        emit_v_group(t8)
        ldp.release()

        # ---- Phase 2: attention per head ----------------------------------
        expp = ctx.enter_context(tc.tile_pool(name="expp", bufs=3))
        rpool = ctx.enter_context(tc.tile_pool(name="rpool", bufs=2))
        oupool = ctx.enter_context(tc.tile_pool(name="oupool", bufs=3))
        with tc.tile_pool(name="psum2", bufs=1, space="PSUM") as pp2:

            def emit_e_chunk(h, kc, junk=False):
                """E^T block (k-chunk kc) for head h -> exp tile in SBUF.

                junk=True emits the matmuls but no exp and returns None: used
                at the end of the attention phase purely to keep the PE's HAM
                activity monitor warm through the proj-phase transition."""
                pe = pp2.tile([128, N], F32, tag="e", bufs=2, name=f"pe_{h}_{kc}")
                for qc in range(2):
                    nc.tensor.matmul(
                        pe[:, qc * 512 : (qc + 1) * 512],
                        KT_sb[:, h, kc * 128 : (kc + 1) * 128],
                        QT_sb[:, h, qc * 512 : (qc + 1) * 512],
                        start=True,
                        stop=True,
                    )
                if junk:
                    return None
                ex = expp.tile([128, N], F32R, tag="ex", name=f"ex_{h}_{kc}")
                nc.scalar.activation(ex[:], pe[:], Exp)
                return ex

            # software pipeline: E-matmuls run one chunk ahead of AV-matmuls so
            # the PE always has exp-independent work while ACT computes exp.
            ex_next = emit_e_chunk(0, 0)
            for h in range(HPC):
                p3, half = divmod(h, 2)
                off = 64 * half
                po = pp2.tile([128, N], F32, tag="o", bufs=2, name=f"po_{h}")
                for kc in range(8):
                    ex = ex_next
                    nh, nkc = (h, kc + 1) if kc < 7 else (h + 1, 0)
                    if nh < HPC:
                        ex_next = emit_e_chunk(nh, nkc)
                    else:
                        # keepalive: dense PE work through the pipeline drain
                        emit_e_chunk(HPC - 1, nkc if kc < 7 else 0, junk=True)
                    for qc in range(2):
                        nc.tensor.matmul(
                            po[:, qc * 512 : (qc + 1) * 512],
                            V_sb[:, kc, h * 65 : h * 65 + 128],
                            ex[:, qc * 512 : (qc + 1) * 512],
                            start=(kc == 0),
                            stop=(kc == 7),
                        )
                # softmax normalization: row 64 of po holds sum_k exp(E^T).
                # Copy the raw block out of PSUM first so the po banks free up
                # for the next head's AV accumulation, then normalize from
                # SBUF off the critical path.  approx-fast reciprocal (~4e-6
                # rel err) is far below the TF32 matmul noise; it misreads
                # nonzero partition offsets on HW, so the sums row gets its
                # own partition-0 copy.
                ou = oupool.tile([65, N], F32, tag="ou", name=f"ou_{h}")
                nc.vector.tensor_copy(ou[:], po[0:65, :])
                ss = rpool.tile([1, N], F32, tag="ss")
                nc.vector.tensor_copy(ss[:], ou[64:65, :])
                rs = rpool.tile([1, N], F32, tag="rs")
                nc.vector.reciprocal_approx_fast(rs[:], ss[:])
                rb = rpool.tile([64, N], F32, tag="rb")
                nc.gpsimd.partition_broadcast(rb[:], rs[:])
                nc.vector.tensor_mul(
                    OT_sb[off : off + 64, p3, :], ou[0:64, :], rb[:]
                )
            # bridge the last head's normalization chain (PE would idle ~5us
            # before the proj phase's first matmul, re-engaging the throttle)
            for w in range(10):
                emit_e_chunk(HPC - 1, w % 8, junk=True)

        # ---- Phase 3: output projection -----------------------------------
        outp = ctx.enter_context(tc.tile_pool(name="outp", bufs=3))
        with tc.tile_pool(name="psum3", bufs=1, space="PSUM") as pp3:
            for qc8 in range(8):
                pso = pp3.tile([128, EMB], F32, tag="p", bufs=3)
                for n0, n1 in ((0, 512), (512, 768)):
                    for ci, c in enumerate((3, 0, 1, 2)):
                        nc.tensor.matmul(
                            pso[:, n0:n1],
                            OT_sb[:, c, qc8 * 128 : (qc8 + 1) * 128],
                            Wp_sb[:, c, n0:n1],
                            start=(ci == 0),
                            stop=(ci == 3),
                        )
                ot = outp.tile([128, EMB], F32, tag="out")
                if qc8 % 2 == 0:
                    nc.vector.tensor_copy(ot[:], pso[:])
                else:
                    nc.scalar.copy(ot[:], pso[:])
                nc.sync.dma_start(out[qc8 * 128 : (qc8 + 1) * 128, :], ot[:])

    nc.compile()
    return nc


def build_in_maps(x, Wqkv, bqkv, Wproj, bproj):
    x = np.asarray(x, dtype=np.float32)
    Wqkv = np.asarray(Wqkv, dtype=np.float32)
    bqkv = np.asarray(bqkv, dtype=np.float32)
    Wproj = np.asarray(Wproj, dtype=np.float32)
    bproj = np.asarray(bproj, dtype=np.float32)

    s = 1.0 / math.sqrt(EMB)
    cols = np.arange(3 * EMB).reshape(HEADS, DH, 3)  # (h, d, qkv) col index map
    in_maps = []
    for c in range(N_CORES):
        b, g = divmod(c, 2)
        hsl = slice(g * HPC, (g + 1) * HPC)
        qcols = cols[hsl, :, 0].reshape(-1)
        kcols = cols[hsl, :, 1].reshape(-1)
        vcols = cols[hsl, :, 2]  # [HPC, DH]

        xT_a = np.zeros((KC * 128, N), np.float32)
        xT_a[:EMB] = x[b].T
        xT_a[EMB] = 1.0

        Wq_a = np.zeros((KC * 128, 384), np.float32)
        Wq_a[:EMB] = Wqkv[:, qcols]
        Wq_a[EMB] = bqkv[qcols]
        Wk_a = np.zeros((KC * 128, 384), np.float32)
        Wk_a[:EMB] = Wqkv[:, kcols]
        Wk_a[EMB] = bqkv[kcols]

        Wv_a = np.zeros((KC * 128, HPC * 65), np.float32)
        for j in range(HPC):
            Wv_a[:EMB, j * 65 : j * 65 + DH] = Wqkv[:, vcols[j]]
            Wv_a[EMB, j * 65 : j * 65 + DH] = bqkv[vcols[j]]
            Wv_a[EMB, j * 65 + DH] = 1.0

        Wp_a = np.zeros((4 * 128, EMB), np.float32)
        Wp_a[:384] = Wproj[g * 384 : (g + 1) * 384] * s
        if g == 0:
            Wp_a[384] = bproj

        in_maps.append({"xT": xT_a, "Wq": Wq_a, "Wk": Wk_a, "Wv": Wv_a, "Wp": Wp_a})
    return in_maps


_NC_CACHE = None


def _get_program():
    global _NC_CACHE
    if _NC_CACHE is None:
        _NC_CACHE = build_program()
    return _NC_CACHE


def kernel(x, Wqkv, bqkv, Wproj, bproj, **_kwargs):
    nc = _get_program()
    in_maps = build_in_maps(x, Wqkv, bqkv, Wproj, bproj)
    res = run_bass_kernel_spmd(nc, in_maps, list(range(N_CORES))).results
    out = np.empty((B, N, EMB), np.float32)
    for b in range(B):
        out[b] = res[2 * b]["out"] + res[2 * b + 1]["out"]
    return out


# revision 21
# speedup vs baseline: 1.1571x; 1.0032x over previous
"""
Multi-head attention (b=4, n=1024, e=768, h=12, dh=64) on 8 trn2 NeuronCores.

Sharding: (batch, head-group) -> core.  Core c handles batch b=c//2 and head
group g=c%2 (6 of the 12 heads).  Each core computes QKV projection for its
heads, attention, and a row-parallel slice of the output projection, producing
a partial [1024, 768] output.  The host sums the two partials per batch
(the row-parallel all-reduce) during unsharding.

All biases ride an augmented contraction row: inputs are padded to 896 (=7*128)
contraction rows where row 768 of xT is ones and row 768 of each weight holds
the bias (rows 769+ are zero).  The softmax denominator comes from an extra
ones-column appended to each head's V (column 64 of each 128-wide head slot),
so the column-sum of exp(E) falls out of the same PE matmul that computes A@V.
The softmax 1/sqrt(768) scale is folded into Wproj on the host.

Matmuls run as float32r (TF32 fast path on the PE).  Attention matmuls are
zero-padded to full 128x128 stationary shapes: half-array matmuls (K=64 E,
M=65 AV) keep the PE's HAM activity monitor below its warm threshold and the
whole attention phase gets clocked at 1.2 GHz instead of 2.4.
"""

import math
from contextlib import ExitStack

import numpy as np

import concourse.bass as bass  # noqa: F401
import concourse.mybir as mybir
import concourse.tile as tile
from concourse import bacc
from concourse.bass_utils import run_bass_kernel_spmd

EMB = 768
HEADS = 12
DH = 64
N = 1024
B = 4
HPC = 6  # heads per core
KC = 7  # contraction chunks (896 = 7*128) for the bias-augmented qkv matmuls
VW = HPC * 65 + 63  # V slots overlap: lhsT reads 128 cols from slot h*65
F32 = mybir.dt.float32
F32R = mybir.dt.float32r

N_CORES = 8


def build_program():
    nc = bacc.Bacc("TRN2", target_bir_lowering=False, debug=False, num_devices=N_CORES)

    xT = nc.dram_tensor("xT", [KC * 128, N], F32R, kind="ExternalInput").ap()
    Wq = nc.dram_tensor("Wq", [KC * 128, 384], F32R, kind="ExternalInput").ap()
    Wk = nc.dram_tensor("Wk", [KC * 128, 384], F32R, kind="ExternalInput").ap()
    Wv = nc.dram_tensor("Wv", [KC * 128, HPC * 65], F32R, kind="ExternalInput").ap()
    Wp = nc.dram_tensor("Wp", [4 * 128, EMB], F32R, kind="ExternalInput").ap()
    out = nc.dram_tensor("out", [N, EMB], F32, kind="ExternalOutput").ap()

    Exp = mybir.ActivationFunctionType.Exp

    with tile.TileContext(nc) as tc, ExitStack() as ctx:
        const = ctx.enter_context(tc.tile_pool(name="const", bufs=1))
        ldp = tc.alloc_tile_pool(name="ldp", bufs=1)

        # Phase-1-only SBUF tensors (pool released after the QKV projections)
        xT_sb = ldp.tile([128, KC, N], F32R)
        Wq_sb = ldp.tile([128, KC, 384], F32R)
        Wk_sb = ldp.tile([128, KC, 384], F32R)
        Wv_sb = ldp.tile([128, KC, HPC * 65], F32R)
        # Resident
        Wp_sb = const.tile([128, 4, EMB], F32R)
        # warmup operand, built by the first two DVE ops so the PE can start
        # its HAM-warmup junk matmuls ~3us in (a DMA-fed warm tile only lands
        # at ~10us: queue bring-up dominates)
        warm_f32 = const.tile([128, 640], F32)
        nc.vector.memset(warm_f32[:], 0.125)
        warm_sb = const.tile([128, 640], F32R)
        nc.vector.tensor_copy(warm_sb[:], warm_f32[:])
        # per-chunk DMAs so QKV matmuls can start before the full load lands
        xT_r = xT.rearrange("(c p) n -> p c n", p=128)
        Wq_r = Wq.rearrange("(c p) n -> p c n", p=128)
        Wk_r = Wk.rearrange("(c p) n -> p c n", p=128)
        Wv_r = Wv.rearrange("(c p) n -> p c n", p=128)
        for c in range(KC):
            nc.sync.dma_start(xT_sb[:, c, :], xT_r[:, c, :])
            nc.sync.dma_start(Wq_sb[:, c, :], Wq_r[:, c, :])
            nc.sync.dma_start(Wk_sb[:, c, :], Wk_r[:, c, :])
        for c in range(KC):
            nc.sync.dma_start(Wv_sb[:, c, :], Wv_r[:, c, :])
        nc.sync.dma_start(Wp_sb[:], Wp.rearrange("(c p) n -> p c n", p=128))

        # Per-head padded Q^T/K^T: head h in partitions 0..63, zeros in 64..127
        # so attention matmuls present full 128x128 stationary shapes to the PE
        # (half-array shapes keep the HAM throttle engaged).
        QT_sb = const.tile([128, HPC, N], F32R)
        KT_sb = const.tile([128, HPC, N], F32R)
        V_sb = const.tile([128, 8, VW], F32R)  # V natural, 8 token chunks
        OT_pad_marker = None  # (V tails + QT/KT pads via DVE below)
        OT_sb = const.tile([128, 4, N], F32R)  # scaled O^T; chunk 3 = [ones; zeros]

        # Zero the padded partitions of QT/KT (both sides must be real zeros:
        # 0 * NaN-garbage would still poison the contraction).  Rows 769.. of
        # xT are zeros by construction, so DMA them in instead of burning DVE
        # time.  The V slot padding columns only feed PSUM rows 65..127, which
        # are never read, so they can stay uninitialized.
        ones_scratch = const.tile([128, N], F32)
        nc.vector.memset(ones_scratch[:], 0.0)
        nc.vector.memset(ones_scratch[0:1, :], 1.0)
        nc.vector.tensor_copy(OT_sb[:, 3, :], ones_scratch[:])
        for h in range(HPC):
            nc.vector.tensor_copy(QT_sb[64:128, h, :], ones_scratch[64:128, :])
            nc.vector.tensor_copy(KT_sb[64:128, h, :], ones_scratch[64:128, :])
        for t8 in range(8):
            nc.vector.tensor_copy(V_sb[:, t8, HPC * 65 :], ones_scratch[:, 0:63])

        # ---- Phase 1: QKV projections -------------------------------------
        with tc.tile_pool(name="psum1", bufs=1, space="PSUM") as pp1:
            # warmup junk matmuls: the PE would otherwise idle for ~12us while
            # the input DMAs land, leaving the HAM clock-gate cold for the
            # first half of the QKV phase
            for w in range(20):
                pw = pp1.tile([128, 512], F32, tag="warm", bufs=1, name=f"pw_{w}")
                nc.tensor.matmul(
                    pw[:], warm_sb[:, 0:128], warm_sb[:, 128:640],
                    start=True, stop=True,
                )
            def emit_v_group(t8):
                psv = pp1.tile([128, HPC * 65], F32, tag="v", bufs=2, name=f"psv_{t8}")
                for c in range(KC):
                    nc.tensor.matmul(
                        psv[:],
                        xT_sb[:, c, t8 * 128 : (t8 + 1) * 128],
                        Wv_sb[:, c, :],
                        start=(c == 0),
                        stop=(c == KC - 1),
                    )
                nc.vector.tensor_copy(V_sb[:, t8, 0 : HPC * 65], psv[:])

            for p3 in range(3):  # head-pair M chunks
                for qc in range(2):
                    psq = pp1.tile([128, 512], F32, tag="qk", bufs=3)
                    for c in range(KC):
                        nc.tensor.matmul(
                            psq[:],
                            Wq_sb[:, c, p3 * 128 : (p3 + 1) * 128],
                            xT_sb[:, c, qc * 512 : (qc + 1) * 512],
                            start=(c == 0),
                            stop=(c == KC - 1),
                        )
                    nc.vector.tensor_copy(
                        QT_sb[0:64, 2 * p3, qc * 512 : (qc + 1) * 512],
                        psq[0:64, :],
                    )
                    nc.scalar.copy(
                        QT_sb[0:64, 2 * p3 + 1, qc * 512 : (qc + 1) * 512],
                        psq[64:128, :],
                    )
                    psk = pp1.tile([128, 512], F32, tag="qk", bufs=3)
                    for c in range(KC):
                        nc.tensor.matmul(
                            psk[:],
                            Wk_sb[:, c, p3 * 128 : (p3 + 1) * 128],
                            xT_sb[:, c, qc * 512 : (qc + 1) * 512],
                            start=(c == 0),
                            stop=(c == KC - 1),
                        )
                    nc.vector.tensor_copy(
                        KT_sb[0:64, 2 * p3, qc * 512 : (qc + 1) * 512],
                        psk[0:64, :],
                    )
                    nc.scalar.copy(
                        KT_sb[0:64, 2 * p3 + 1, qc * 512 : (qc + 1) * 512],
                        psk[64:128, :],
                    )
                if p3 == 0:  # V right after head pair 0 so attention can start
                    for t8 in range(8):
                        emit_v_group(t8)
        ldp.release()

        # ---- Phase 2: attention per head ----------------------------------
        expp = ctx.enter_context(tc.tile_pool(name="expp", bufs=4))
        rpool = ctx.enter_context(tc.tile_pool(name="rpool", bufs=2))
        oupool = ctx.enter_context(tc.tile_pool(name="oupool", bufs=3))
        with tc.tile_pool(name="psum2", bufs=1, space="PSUM") as pp2:

            def emit_e_chunk(h, kc, junk=False):
                """E^T block (k-chunk kc) for head h -> exp tile in SBUF.

                junk=True emits the matmuls but no exp and returns None: used
                at the end of the attention phase purely to keep the PE's HAM
                activity monitor warm through the proj-phase transition."""
                pe = pp2.tile([128, N], F32, tag="e", bufs=2, name=f"pe_{h}_{kc}")
                for qc in range(2):
                    nc.tensor.matmul(
                        pe[:, qc * 512 : (qc + 1) * 512],
                        KT_sb[:, h, kc * 128 : (kc + 1) * 128],
                        QT_sb[:, h, qc * 512 : (qc + 1) * 512],
                        start=True,
                        stop=True,
                    )
                if junk:
                    return None
                ex = expp.tile([128, N], F32R, tag="ex", name=f"ex_{h}_{kc}")
                nc.scalar.activation(ex[:], pe[:], Exp)
                return ex

            # software pipeline: E-matmuls run one chunk ahead of AV-matmuls so
            # the PE always has exp-independent work while ACT computes exp.
            ex_next = emit_e_chunk(0, 0)
            for h in range(HPC):
                p3, half = divmod(h, 2)
                off = 64 * half
                po = pp2.tile([128, N], F32, tag="o", bufs=2, name=f"po_{h}")
                for kc in range(8):
                    ex = ex_next
                    nh, nkc = (h, kc + 1) if kc < 7 else (h + 1, 0)
                    if nh < HPC:
                        ex_next = emit_e_chunk(nh, nkc)
                    else:
                        # keepalive: dense PE work through the pipeline drain
                        emit_e_chunk(HPC - 1, nkc if kc < 7 else 0, junk=True)
                    for qc in range(2):
                        nc.tensor.matmul(
                            po[:, qc * 512 : (qc + 1) * 512],
                            V_sb[:, kc, h * 65 : h * 65 + 128],
                            ex[:, qc * 512 : (qc + 1) * 512],
                            start=(kc == 0),
                            stop=(kc == 7),
                        )
                # softmax normalization: row 64 of po holds sum_k exp(E^T).
                # Copy the raw block out of PSUM first so the po banks free up
                # for the next head's AV accumulation, then normalize from
                # SBUF off the critical path.  approx-fast reciprocal (~4e-6
                # rel err) is far below the TF32 matmul noise; it misreads
                # nonzero partition offsets on HW, so the sums row gets its
                # own partition-0 copy.
                ou = oupool.tile([65, N], F32, tag="ou", name=f"ou_{h}")
                nc.vector.tensor_copy(ou[:], po[0:65, :])
                ss = rpool.tile([1, N], F32, tag="ss")
                nc.vector.tensor_copy(ss[:], ou[64:65, :])
                rs = rpool.tile([1, N], F32, tag="rs")
                nc.vector.reciprocal_approx_fast(rs[:], ss[:])
                rb = rpool.tile([64, N], F32, tag="rb")
                nc.gpsimd.partition_broadcast(rb[:], rs[:])
                nc.vector.tensor_mul(
                    OT_sb[off : off + 64, p3, :], ou[0:64, :], rb[:]
                )
            # bridge the last head's normalization chain (PE would idle ~5us
            # before the proj phase's first matmul, re-engaging the throttle)
            for w in range(10):
                emit_e_chunk(HPC - 1, w % 8, junk=True)

        # ---- Phase 3: output projection -----------------------------------
        outp = ctx.enter_context(tc.tile_pool(name="outp", bufs=3))
        with tc.tile_pool(name="psum3", bufs=1, space="PSUM") as pp3:
            for qc8 in range(8):
                pso = pp3.tile([128, EMB], F32, tag="p", bufs=3)
                for n0, n1 in ((0, 512), (512, 768)):
                    for ci, c in enumerate((3, 0, 1, 2)):
                        nc.tensor.matmul(
                            pso[:, n0:n1],
                            OT_sb[:, c, qc8 * 128 : (qc8 + 1) * 128],
                            Wp_sb[:, c, n0:n1],
                            start=(ci == 0),
                            stop=(ci == 3),
                        )
                ot = outp.tile([128, EMB], F32, tag="out")
                if qc8 % 2 == 0:
                    nc.vector.tensor_copy(ot[:], pso[:])
                else:
                    nc.scalar.copy(ot[:], pso[:])
                nc.sync.dma_start(out[qc8 * 128 : (qc8 + 1) * 128, :], ot[:])

    nc.compile()
    return nc


def build_in_maps(x, Wqkv, bqkv, Wproj, bproj):
    x = np.asarray(x, dtype=np.float32)
    Wqkv = np.asarray(Wqkv, dtype=np.float32)
    bqkv = np.asarray(bqkv, dtype=np.float32)
    Wproj = np.asarray(Wproj, dtype=np.float32)
    bproj = np.asarray(bproj, dtype=np.float32)

    s = 1.0 / math.sqrt(EMB)
    cols = np.arange(3 * EMB).reshape(HEADS, DH, 3)  # (h, d, qkv) col index map
    in_maps = []
    for c in range(N_CORES):
        b, g = divmod(c, 2)
        hsl = slice(g * HPC, (g + 1) * HPC)
        qcols = cols[hsl, :, 0].reshape(-1)
        kcols = cols[hsl, :, 1].reshape(-1)
        vcols = cols[hsl, :, 2]  # [HPC, DH]

        xT_a = np.zeros((KC * 128, N), np.float32)
        xT_a[:EMB] = x[b].T
        xT_a[EMB] = 1.0

        Wq_a = np.zeros((KC * 128, 384), np.float32)
        Wq_a[:EMB] = Wqkv[:, qcols]
        Wq_a[EMB] = bqkv[qcols]
        Wk_a = np.zeros((KC * 128, 384), np.float32)
        Wk_a[:EMB] = Wqkv[:, kcols]
        Wk_a[EMB] = bqkv[kcols]

        Wv_a = np.zeros((KC * 128, HPC * 65), np.float32)
        for j in range(HPC):
            Wv_a[:EMB, j * 65 : j * 65 + DH] = Wqkv[:, vcols[j]]
            Wv_a[EMB, j * 65 : j * 65 + DH] = bqkv[vcols[j]]
            Wv_a[EMB, j * 65 + DH] = 1.0

        Wp_a = np.zeros((4 * 128, EMB), np.float32)
        Wp_a[:384] = Wproj[g * 384 : (g + 1) * 384] * s
        if g == 0:
            Wp_a[384] = bproj

        in_maps.append({"xT": xT_a, "Wq": Wq_a, "Wk": Wk_a, "Wv": Wv_a, "Wp": Wp_a})
    return in_maps


_NC_CACHE = None


def _get_program():
    global _NC_CACHE
    if _NC_CACHE is None:
        _NC_CACHE = build_program()
    return _NC_CACHE


def kernel(x, Wqkv, bqkv, Wproj, bproj, **_kwargs):
    nc = _get_program()
    in_maps = build_in_maps(x, Wqkv, bqkv, Wproj, bproj)
    res = run_bass_kernel_spmd(nc, in_maps, list(range(N_CORES))).results
    out = np.empty((B, N, EMB), np.float32)
    for b in range(B):
        out[b] = res[2 * b]["out"] + res[2 * b + 1]["out"]
    return out


# revision 22
# speedup vs baseline: 1.1926x; 1.0307x over previous
"""
Multi-head attention (b=4, n=1024, e=768, h=12, dh=64) on 8 trn2 NeuronCores.

Sharding: (batch, head-group) -> core.  Core c handles batch b=c//2 and head
group g=c%2 (6 of the 12 heads).  Each core computes QKV projection for its
heads, attention, and a row-parallel slice of the output projection, producing
a partial [1024, 768] output.  The host sums the two partials per batch
(the row-parallel all-reduce) during unsharding.

All biases ride an augmented contraction row: inputs are padded to 896 (=7*128)
contraction rows where row 768 of xT is ones and row 768 of each weight holds
the bias (rows 769+ are zero).  The softmax denominator comes from an extra
ones-column appended to each head's V (column 64 of each 128-wide head slot),
so the column-sum of exp(E) falls out of the same PE matmul that computes A@V.
The softmax 1/sqrt(768) scale is folded into Wproj on the host.

Matmuls run as float32r (TF32 fast path on the PE).  Attention matmuls are
zero-padded to full 128x128 stationary shapes: half-array matmuls (K=64 E,
M=65 AV) keep the PE's HAM activity monitor below its warm threshold and the
whole attention phase gets clocked at 1.2 GHz instead of 2.4.
"""

import math
from contextlib import ExitStack

import numpy as np

import concourse.bass as bass  # noqa: F401
import concourse.mybir as mybir
import concourse.tile as tile
from concourse import bacc
from concourse.bass_utils import run_bass_kernel_spmd

EMB = 768
HEADS = 12
DH = 64
N = 1024
B = 4
HPC = 6  # heads per core
KC = 7  # contraction chunks (896 = 7*128) for the bias-augmented qkv matmuls
VW = HPC * 65 + 63  # V slots overlap: lhsT reads 128 cols from slot h*65
F32 = mybir.dt.float32
F32R = mybir.dt.float32r

N_CORES = 8


def build_program():
    nc = bacc.Bacc("TRN2", target_bir_lowering=False, debug=False, num_devices=N_CORES)

    xT = nc.dram_tensor("xT", [KC * 128, N], F32R, kind="ExternalInput").ap()
    Wq = nc.dram_tensor("Wq", [KC * 128, 384], F32R, kind="ExternalInput").ap()
    Wk = nc.dram_tensor("Wk", [KC * 128, 384], F32R, kind="ExternalInput").ap()
    Wv = nc.dram_tensor("Wv", [KC * 128, HPC * 65], F32R, kind="ExternalInput").ap()
    Wp = nc.dram_tensor("Wp", [4 * 128, EMB], F32R, kind="ExternalInput").ap()
    out = nc.dram_tensor("out", [N, EMB], F32, kind="ExternalOutput").ap()

    Exp = mybir.ActivationFunctionType.Exp

    with tile.TileContext(nc) as tc, ExitStack() as ctx:
        const = ctx.enter_context(tc.tile_pool(name="const", bufs=1))
        ldp = tc.alloc_tile_pool(name="ldp", bufs=1)

        # Phase-1-only SBUF tensors (pool released after the QKV projections)
        xT_sb = ldp.tile([128, KC, N], F32R)
        Wq_sb = ldp.tile([128, KC, 384], F32R)
        Wk_sb = ldp.tile([128, KC, 384], F32R)
        Wv_sb = ldp.tile([128, KC, HPC * 65], F32R)
        # Resident
        Wp_sb = const.tile([128, 4, EMB], F32R)
        # warmup operand, built by the first two DVE ops so the PE can start
        # its HAM-warmup junk matmuls ~3us in (a DMA-fed warm tile only lands
        # at ~10us: queue bring-up dominates)
        warm_f32 = const.tile([128, 640], F32)
        nc.vector.memset(warm_f32[:], 0.125)
        warm_sb = const.tile([128, 640], F32R)
        nc.vector.tensor_copy(warm_sb[:], warm_f32[:])
        # per-chunk DMAs so QKV matmuls can start before the full load lands
        xT_r = xT.rearrange("(c p) n -> p c n", p=128)
        Wq_r = Wq.rearrange("(c p) n -> p c n", p=128)
        Wk_r = Wk.rearrange("(c p) n -> p c n", p=128)
        Wv_r = Wv.rearrange("(c p) n -> p c n", p=128)
        for c in range(KC):
            nc.sync.dma_start(xT_sb[:, c, :], xT_r[:, c, :])
            nc.sync.dma_start(Wq_sb[:, c, :], Wq_r[:, c, :])
            nc.sync.dma_start(Wk_sb[:, c, :], Wk_r[:, c, :])
        for c in range(KC):
            nc.sync.dma_start(Wv_sb[:, c, :], Wv_r[:, c, :])
        nc.sync.dma_start(Wp_sb[:], Wp.rearrange("(c p) n -> p c n", p=128))

        # Per-head padded Q^T/K^T: head h in partitions 0..63, zeros in 64..127
        # so attention matmuls present full 128x128 stationary shapes to the PE
        # (half-array shapes keep the HAM throttle engaged).
        QT_sb = const.tile([128, HPC, N], F32R)
        KT_sb = const.tile([128, HPC, N], F32R)
        V_sb = const.tile([128, 8, VW], F32R)  # V natural, 8 token chunks
        OT_pad_marker = None  # (V tails + QT/KT pads via DVE below)
        OT_sb = const.tile([128, 4, N], F32R)  # scaled O^T; chunk 3 = [ones; zeros]

        # Zero the padded partitions of QT/KT (both sides must be real zeros:
        # 0 * NaN-garbage would still poison the contraction).  Rows 769.. of
        # xT are zeros by construction, so DMA them in instead of burning DVE
        # time.  The V slot padding columns only feed PSUM rows 65..127, which
        # are never read, so they can stay uninitialized.
        ones_scratch = const.tile([128, N], F32)
        nc.vector.memset(ones_scratch[:], 0.0)
        nc.vector.memset(ones_scratch[0:1, :], 1.0)
        nc.vector.tensor_copy(OT_sb[:, 3, :], ones_scratch[:])
        for h in range(HPC):
            nc.vector.tensor_copy(QT_sb[64:128, h, :], ones_scratch[64:128, :])
            nc.vector.tensor_copy(KT_sb[64:128, h, :], ones_scratch[64:128, :])
        for t8 in range(8):
            nc.vector.tensor_copy(V_sb[:, t8, HPC * 65 :], ones_scratch[:, 0:63])

        # ---- Phase 1: QKV projections -------------------------------------
        with tc.tile_pool(name="psum1", bufs=1, space="PSUM") as pp1:
            # warmup junk matmuls: the PE would otherwise idle for ~10us while
            # the input DMAs land, leaving the HAM clock-gate cold for the
            # first half of the QKV phase
            for w in range(8):
                pw = pp1.tile([128, 512], F32, tag="qk", bufs=6, name=f"pw_{w}")
                nc.tensor.matmul(
                    pw[:], warm_sb[:, 0:128], warm_sb[:, 128:640],
                    start=True, stop=True,
                )
            def emit_v_group(t8):
                psv = pp1.tile([128, HPC * 65], F32, tag="v", bufs=2, name=f"psv_{t8}")
                for c in range(KC):
                    nc.tensor.matmul(
                        psv[:],
                        xT_sb[:, c, t8 * 128 : (t8 + 1) * 128],
                        Wv_sb[:, c, :],
                        start=(c == 0),
                        stop=(c == KC - 1),
                    )
                nc.vector.tensor_copy(V_sb[:, t8, 0 : HPC * 65], psv[:])

            groups = [(p3, qc) for p3 in range(3) for qc in range(2)]
            for W_sb, T_sb in ((Wq_sb, QT_sb), (Wk_sb, KT_sb)):
                gt = [
                    pp1.tile([128, 512], F32, tag="qk", bufs=6, name=f"g_{i}")
                    for i in range(6)
                ]
                for c in range(KC):
                    for i, (p3, qc) in enumerate(groups):
                        nc.tensor.matmul(
                            gt[i],
                            W_sb[:, c, p3 * 128 : (p3 + 1) * 128],
                            xT_sb[:, c, qc * 512 : (qc + 1) * 512],
                            start=(c == 0),
                            stop=(c == KC - 1),
                        )
                for i, (p3, qc) in enumerate(groups):
                    nc.vector.tensor_copy(
                        T_sb[0:64, 2 * p3, qc * 512 : (qc + 1) * 512],
                        gt[i][0:64, :],
                    )
                    nc.scalar.copy(
                        T_sb[0:64, 2 * p3 + 1, qc * 512 : (qc + 1) * 512],
                        gt[i][64:128, :],
                    )
            if True:
                for t8 in range(8):
                    emit_v_group(t8)
        ldp.release()

        # ---- Phase 2: attention per head ----------------------------------
        expp = ctx.enter_context(tc.tile_pool(name="expp", bufs=4))
        rpool = ctx.enter_context(tc.tile_pool(name="rpool", bufs=2))
        oupool = ctx.enter_context(tc.tile_pool(name="oupool", bufs=3))
        with tc.tile_pool(name="psum2", bufs=1, space="PSUM") as pp2:

            def emit_e_chunk(h, kc, junk=False):
                """E^T block (k-chunk kc) for head h -> exp tile in SBUF.

                junk=True emits the matmuls but no exp and returns None: used
                at the end of the attention phase purely to keep the PE's HAM
                activity monitor warm through the proj-phase transition."""
                pe = pp2.tile([128, N], F32, tag="e", bufs=2, name=f"pe_{h}_{kc}")
                for qc in range(2):
                    nc.tensor.matmul(
                        pe[:, qc * 512 : (qc + 1) * 512],
                        KT_sb[:, h, kc * 128 : (kc + 1) * 128],
                        QT_sb[:, h, qc * 512 : (qc + 1) * 512],
                        start=True,
                        stop=True,
                    )
                if junk:
                    return None
                ex = expp.tile([128, N], F32R, tag="ex", name=f"ex_{h}_{kc}")
                nc.scalar.activation(ex[:], pe[:], Exp)
                return ex

            # software pipeline: E-matmuls run one chunk ahead of AV-matmuls so
            # the PE always has exp-independent work while ACT computes exp.
            ex_next = emit_e_chunk(0, 0)
            for h in range(HPC):
                p3, half = divmod(h, 2)
                off = 64 * half
                po = pp2.tile([128, N], F32, tag="o", bufs=2, name=f"po_{h}")
                for kc in range(8):
                    ex = ex_next
                    nh, nkc = (h, kc + 1) if kc < 7 else (h + 1, 0)
                    if nh < HPC:
                        ex_next = emit_e_chunk(nh, nkc)
                    else:
                        # keepalive: dense PE work through the pipeline drain
                        emit_e_chunk(HPC - 1, nkc if kc < 7 else 0, junk=True)
                    for qc in range(2):
                        nc.tensor.matmul(
                            po[:, qc * 512 : (qc + 1) * 512],
                            V_sb[:, kc, h * 65 : h * 65 + 128],
                            ex[:, qc * 512 : (qc + 1) * 512],
                            start=(kc == 0),
                            stop=(kc == 7),
                        )
                # softmax normalization: row 64 of po holds sum_k exp(E^T).
                # Copy the raw block out of PSUM first so the po banks free up
                # for the next head's AV accumulation, then normalize from
                # SBUF off the critical path.  approx-fast reciprocal (~4e-6
                # rel err) is far below the TF32 matmul noise; it misreads
                # nonzero partition offsets on HW, so the sums row gets its
                # own partition-0 copy.
                ou = oupool.tile([65, N], F32, tag="ou", name=f"ou_{h}")
                nc.vector.tensor_copy(ou[:], po[0:65, :])
                ss = rpool.tile([1, N], F32, tag="ss")
                nc.vector.tensor_copy(ss[:], ou[64:65, :])
                rs = rpool.tile([1, N], F32, tag="rs")
                nc.vector.reciprocal_approx_fast(rs[:], ss[:])
                rb = rpool.tile([64, N], F32, tag="rb")
                nc.gpsimd.partition_broadcast(rb[:], rs[:])
                nc.vector.tensor_mul(
                    OT_sb[off : off + 64, p3, :], ou[0:64, :], rb[:]
                )
            # bridge the last head's normalization chain (PE would idle ~5us
            # before the proj phase's first matmul, re-engaging the throttle)
            for w in range(10):
                emit_e_chunk(HPC - 1, w % 8, junk=True)

        # ---- Phase 3: output projection -----------------------------------
        outp = ctx.enter_context(tc.tile_pool(name="outp", bufs=3))
        with tc.tile_pool(name="psum3", bufs=1, space="PSUM") as pp3:
            for qc8 in range(8):
                pso = pp3.tile([128, EMB], F32, tag="p", bufs=3)
                for n0, n1 in ((0, 512), (512, 768)):
                    for ci, c in enumerate((3, 0, 1, 2)):
                        nc.tensor.matmul(
                            pso[:, n0:n1],
                            OT_sb[:, c, qc8 * 128 : (qc8 + 1) * 128],
                            Wp_sb[:, c, n0:n1],
                            start=(ci == 0),
                            stop=(ci == 3),
                        )
                ot = outp.tile([128, EMB], F32, tag="out")
                if qc8 % 2 == 0:
                    nc.vector.tensor_copy(ot[:], pso[:])
                else:
                    nc.scalar.copy(ot[:], pso[:])
                nc.sync.dma_start(out[qc8 * 128 : (qc8 + 1) * 128, :], ot[:])

    nc.compile()
    return nc


def build_in_maps(x, Wqkv, bqkv, Wproj, bproj):
    x = np.asarray(x, dtype=np.float32)
    Wqkv = np.asarray(Wqkv, dtype=np.float32)
    bqkv = np.asarray(bqkv, dtype=np.float32)
    Wproj = np.asarray(Wproj, dtype=np.float32)
    bproj = np.asarray(bproj, dtype=np.float32)

    s = 1.0 / math.sqrt(EMB)
    cols = np.arange(3 * EMB).reshape(HEADS, DH, 3)  # (h, d, qkv) col index map
    in_maps = []
    for c in range(N_CORES):
        b, g = divmod(c, 2)
        hsl = slice(g * HPC, (g + 1) * HPC)
        qcols = cols[hsl, :, 0].reshape(-1)
        kcols = cols[hsl, :, 1].reshape(-1)
        vcols = cols[hsl, :, 2]  # [HPC, DH]

        xT_a = np.zeros((KC * 128, N), np.float32)
        xT_a[:EMB] = x[b].T
        xT_a[EMB] = 1.0

        Wq_a = np.zeros((KC * 128, 384), np.float32)
        Wq_a[:EMB] = Wqkv[:, qcols]
        Wq_a[EMB] = bqkv[qcols]
        Wk_a = np.zeros((KC * 128, 384), np.float32)
        Wk_a[:EMB] = Wqkv[:, kcols]
        Wk_a[EMB] = bqkv[kcols]

        Wv_a = np.zeros((KC * 128, HPC * 65), np.float32)
        for j in range(HPC):
            Wv_a[:EMB, j * 65 : j * 65 + DH] = Wqkv[:, vcols[j]]
            Wv_a[EMB, j * 65 : j * 65 + DH] = bqkv[vcols[j]]
            Wv_a[EMB, j * 65 + DH] = 1.0

        Wp_a = np.zeros((4 * 128, EMB), np.float32)
        Wp_a[:384] = Wproj[g * 384 : (g + 1) * 384] * s
        if g == 0:
            Wp_a[384] = bproj

        in_maps.append({"xT": xT_a, "Wq": Wq_a, "Wk": Wk_a, "Wv": Wv_a, "Wp": Wp_a})
    return in_maps


_NC_CACHE = None


def _get_program():
    global _NC_CACHE
    if _NC_CACHE is None:
        _NC_CACHE = build_program()
    return _NC_CACHE


def kernel(x, Wqkv, bqkv, Wproj, bproj, **_kwargs):
    nc = _get_program()
    in_maps = build_in_maps(x, Wqkv, bqkv, Wproj, bproj)
    res = run_bass_kernel_spmd(nc, in_maps, list(range(N_CORES))).results
    out = np.empty((B, N, EMB), np.float32)
    for b in range(B):
        out[b] = res[2 * b]["out"] + res[2 * b + 1]["out"]
    return out


# revision 23
# speedup vs baseline: 1.2311x; 1.0323x over previous
"""
Multi-head attention (b=4, n=1024, e=768, h=12, dh=64) on 8 trn2 NeuronCores.

Sharding: (batch, head-group) -> core.  Core c handles batch b=c//2 and head
group g=c%2 (6 of the 12 heads).  Each core computes QKV projection for its
heads, attention, and a row-parallel slice of the output projection, producing
a partial [1024, 768] output.  The host sums the two partials per batch
(the row-parallel all-reduce) during unsharding.

All biases ride an augmented contraction row: inputs are padded to 896 (=7*128)
contraction rows where row 768 of xT is ones and row 768 of each weight holds
the bias (rows 769+ are zero).  The softmax denominator comes from an extra
ones-column appended to each head's V (column 64 of each 128-wide head slot),
so the column-sum of exp(E) falls out of the same PE matmul that computes A@V.
The softmax 1/sqrt(768) scale is folded into Wproj on the host.

Matmuls run as float32r (TF32 fast path on the PE).  Attention matmuls are
zero-padded to full 128x128 stationary shapes: half-array matmuls (K=64 E,
M=65 AV) keep the PE's HAM activity monitor below its warm threshold and the
whole attention phase gets clocked at 1.2 GHz instead of 2.4.
"""

import math
from contextlib import ExitStack

import numpy as np

import concourse.bass as bass  # noqa: F401
import concourse.mybir as mybir
import concourse.tile as tile
from concourse import bacc
from concourse.bass_utils import run_bass_kernel_spmd

EMB = 768
HEADS = 12
DH = 64
N = 1024
B = 4
HPC = 6  # heads per core
KC = 7  # contraction chunks (896 = 7*128) for the bias-augmented qkv matmuls
VW = HPC * 65 + 63  # V slots overlap: lhsT reads 128 cols from slot h*65
F32 = mybir.dt.float32
F32R = mybir.dt.float32r

N_CORES = 8


def build_program():
    nc = bacc.Bacc("TRN2", target_bir_lowering=False, debug=False, num_devices=N_CORES)

    xT = nc.dram_tensor("xT", [KC * 128, N], F32R, kind="ExternalInput").ap()
    Wq = nc.dram_tensor("Wq", [KC * 128, 384], F32R, kind="ExternalInput").ap()
    Wk = nc.dram_tensor("Wk", [KC * 128, 384], F32R, kind="ExternalInput").ap()
    Wv = nc.dram_tensor("Wv", [KC * 128, HPC * 65], F32R, kind="ExternalInput").ap()
    Wp = nc.dram_tensor("Wp", [4 * 128, EMB], F32R, kind="ExternalInput").ap()
    out = nc.dram_tensor("out", [N, EMB], F32, kind="ExternalOutput").ap()

    Exp = mybir.ActivationFunctionType.Exp

    with tile.TileContext(nc) as tc, ExitStack() as ctx:
        const = ctx.enter_context(tc.tile_pool(name="const", bufs=1))
        ldp = tc.alloc_tile_pool(name="ldp", bufs=1)

        # Phase-1-only SBUF tensors (pool released after the QKV projections)
        xT_sb = ldp.tile([128, KC, N], F32R)
        Wq_sb = ldp.tile([128, KC, 384], F32R)
        Wk_sb = ldp.tile([128, KC, 384], F32R)
        Wv_sb = ldp.tile([128, KC, HPC * 65], F32R)
        # Resident
        Wp_sb = const.tile([128, 4, EMB], F32R)
        # warmup operand, built by the first two DVE ops so the PE can start
        # its HAM-warmup junk matmuls ~3us in (a DMA-fed warm tile only lands
        # at ~10us: queue bring-up dominates)
        warm_f32 = const.tile([128, 640], F32)
        nc.vector.memset(warm_f32[:], 0.125)
        warm_sb = const.tile([128, 640], F32R)
        nc.vector.tensor_copy(warm_sb[:], warm_f32[:])
        # per-chunk DMAs so QKV matmuls can start before the full load lands
        xT_r = xT.rearrange("(c p) n -> p c n", p=128)
        Wq_r = Wq.rearrange("(c p) n -> p c n", p=128)
        Wk_r = Wk.rearrange("(c p) n -> p c n", p=128)
        Wv_r = Wv.rearrange("(c p) n -> p c n", p=128)
        for c in range(KC):
            nc.sync.dma_start(xT_sb[:, c, :], xT_r[:, c, :])
            nc.sync.dma_start(Wq_sb[:, c, :], Wq_r[:, c, :])
            nc.sync.dma_start(Wk_sb[:, c, :], Wk_r[:, c, :])
            if c >= 3:
                nc.sync.dma_start(Wv_sb[:, c - 3, :], Wv_r[:, c - 3, :])
        for c in range(KC - 3, KC):
            nc.sync.dma_start(Wv_sb[:, c, :], Wv_r[:, c, :])
        nc.sync.dma_start(Wp_sb[:], Wp.rearrange("(c p) n -> p c n", p=128))

        # Per-head padded Q^T/K^T: head h in partitions 0..63, zeros in 64..127
        # so attention matmuls present full 128x128 stationary shapes to the PE
        # (half-array shapes keep the HAM throttle engaged).
        QT_sb = const.tile([128, HPC, N], F32R)
        KT_sb = const.tile([128, HPC, N], F32R)
        V_sb = const.tile([128, 8, VW], F32R)  # V natural, 8 token chunks
        OT_pad_marker = None  # (V tails + QT/KT pads via DVE below)
        OT_sb = const.tile([128, 4, N], F32R)  # scaled O^T; chunk 3 = [ones; zeros]

        # Zero the padded partitions of QT/KT (both sides must be real zeros:
        # 0 * NaN-garbage would still poison the contraction).  Rows 769.. of
        # xT are zeros by construction, so DMA them in instead of burning DVE
        # time.  The V slot padding columns only feed PSUM rows 65..127, which
        # are never read, so they can stay uninitialized.
        ones_scratch = const.tile([128, N], F32)
        nc.vector.memset(ones_scratch[:], 0.0)
        nc.vector.memset(ones_scratch[0:1, :], 1.0)
        nc.vector.tensor_copy(OT_sb[:, 3, :], ones_scratch[:])
        for h in range(HPC):
            nc.vector.tensor_copy(QT_sb[64:128, h, :], ones_scratch[64:128, :])
            nc.vector.tensor_copy(KT_sb[64:128, h, :], ones_scratch[64:128, :])
        for t8 in range(8):
            nc.vector.tensor_copy(V_sb[:, t8, HPC * 65 :], ones_scratch[:, 0:63])

        # ---- Phase 1: QKV projections -------------------------------------
        with tc.tile_pool(name="psum1", bufs=1, space="PSUM") as pp1:
            # warmup junk matmuls: the PE would otherwise idle for ~10us while
            # the input DMAs land, leaving the HAM clock-gate cold for the
            # first half of the QKV phase
            for w in range(8):
                pw = pp1.tile([128, 512], F32, tag="qk", bufs=6, name=f"pw_{w}")
                nc.tensor.matmul(
                    pw[:], warm_sb[:, 0:128], warm_sb[:, 128:640],
                    start=True, stop=True,
                )
            def emit_v_group(t8):
                psv = pp1.tile([128, HPC * 65], F32, tag="v", bufs=2, name=f"psv_{t8}")
                for c in range(KC):
                    nc.tensor.matmul(
                        psv[:],
                        xT_sb[:, c, t8 * 128 : (t8 + 1) * 128],
                        Wv_sb[:, c, :],
                        start=(c == 0),
                        stop=(c == KC - 1),
                    )
                nc.vector.tensor_copy(V_sb[:, t8, 0 : HPC * 65], psv[:])

            groups = [(p3, qc) for p3 in range(3) for qc in range(2)]
            for W_sb, T_sb in ((Wq_sb, QT_sb), (Wk_sb, KT_sb)):
                gt = [
                    pp1.tile([128, 512], F32, tag="qk", bufs=6, name=f"g_{i}")
                    for i in range(6)
                ]
                for c in range(KC):
                    for i, (p3, qc) in enumerate(groups):
                        nc.tensor.matmul(
                            gt[i],
                            W_sb[:, c, p3 * 128 : (p3 + 1) * 128],
                            xT_sb[:, c, qc * 512 : (qc + 1) * 512],
                            start=(c == 0),
                            stop=(c == KC - 1),
                        )
                for i, (p3, qc) in enumerate(groups):
                    nc.vector.tensor_copy(
                        T_sb[0:64, 2 * p3, qc * 512 : (qc + 1) * 512],
                        gt[i][0:64, :],
                    )
                    nc.scalar.copy(
                        T_sb[0:64, 2 * p3 + 1, qc * 512 : (qc + 1) * 512],
                        gt[i][64:128, :],
                    )
            if True:
                for t8 in range(8):
                    emit_v_group(t8)
        ldp.release()

        # ---- Phase 2: attention per head ----------------------------------
        expp = ctx.enter_context(tc.tile_pool(name="expp", bufs=4))
        rpool = ctx.enter_context(tc.tile_pool(name="rpool", bufs=2))
        oupool = ctx.enter_context(tc.tile_pool(name="oupool", bufs=3))
        with tc.tile_pool(name="psum2", bufs=1, space="PSUM") as pp2:

            def emit_e_chunk(h, kc, junk=False):
                """E^T block (k-chunk kc) for head h -> exp tile in SBUF.

                junk=True emits the matmuls but no exp and returns None: used
                at the end of the attention phase purely to keep the PE's HAM
                activity monitor warm through the proj-phase transition."""
                pe = pp2.tile([128, N], F32, tag="e", bufs=2, name=f"pe_{h}_{kc}")
                for qc in range(2):
                    nc.tensor.matmul(
                        pe[:, qc * 512 : (qc + 1) * 512],
                        KT_sb[:, h, kc * 128 : (kc + 1) * 128],
                        QT_sb[:, h, qc * 512 : (qc + 1) * 512],
                        start=True,
                        stop=True,
                    )
                if junk:
                    return None
                ex = expp.tile([128, N], F32R, tag="ex", name=f"ex_{h}_{kc}")
                nc.scalar.activation(ex[:], pe[:], Exp)
                return ex

            # software pipeline: E-matmuls run one chunk ahead of AV-matmuls so
            # the PE always has exp-independent work while ACT computes exp.
            ex_next = emit_e_chunk(0, 0)
            for h in range(HPC):
                p3, half = divmod(h, 2)
                off = 64 * half
                po = pp2.tile([128, N], F32, tag="o", bufs=2, name=f"po_{h}")
                for kc in range(8):
                    ex = ex_next
                    nh, nkc = (h, kc + 1) if kc < 7 else (h + 1, 0)
                    if nh < HPC:
                        ex_next = emit_e_chunk(nh, nkc)
                    else:
                        # keepalive: dense PE work through the pipeline drain
                        emit_e_chunk(HPC - 1, nkc if kc < 7 else 0, junk=True)
                    for qc in range(2):
                        nc.tensor.matmul(
                            po[:, qc * 512 : (qc + 1) * 512],
                            V_sb[:, kc, h * 65 : h * 65 + 128],
                            ex[:, qc * 512 : (qc + 1) * 512],
                            start=(kc == 0),
                            stop=(kc == 7),
                        )
                # softmax normalization: row 64 of po holds sum_k exp(E^T).
                # Copy the raw block out of PSUM first so the po banks free up
                # for the next head's AV accumulation, then normalize from
                # SBUF off the critical path.  approx-fast reciprocal (~4e-6
                # rel err) is far below the TF32 matmul noise; it misreads
                # nonzero partition offsets on HW, so the sums row gets its
                # own partition-0 copy.
                ou = oupool.tile([65, N], F32, tag="ou", name=f"ou_{h}")
                nc.vector.tensor_copy(ou[:], po[0:65, :])
                ss = rpool.tile([1, N], F32, tag="ss")
                nc.vector.tensor_copy(ss[:], ou[64:65, :])
                rs = rpool.tile([1, N], F32, tag="rs")
                nc.vector.reciprocal_approx_fast(rs[:], ss[:])
                rb = rpool.tile([64, N], F32, tag="rb")
                nc.gpsimd.partition_broadcast(rb[:], rs[:])
                nc.vector.tensor_mul(
                    OT_sb[off : off + 64, p3, :], ou[0:64, :], rb[:]
                )
            # bridge the last head's normalization chain (PE would idle ~4us
            # before the proj phase's head-4/5 matmuls, re-engaging the
            # throttle; the c-order (3,0,1,2) proj groups cover part of it)
            for w in range(4):
                emit_e_chunk(HPC - 1, w % 8, junk=True)

        # ---- Phase 3: output projection -----------------------------------
        outp = ctx.enter_context(tc.tile_pool(name="outp", bufs=3))
        with tc.tile_pool(name="psum3", bufs=1, space="PSUM") as pp3:
            for qc8 in range(8):
                pso = pp3.tile([128, EMB], F32, tag="p", bufs=3)
                for n0, n1 in ((0, 512), (512, 768)):
                    for ci, c in enumerate((3, 0, 1, 2)):
                        nc.tensor.matmul(
                            pso[:, n0:n1],
                            OT_sb[:, c, qc8 * 128 : (qc8 + 1) * 128],
                            Wp_sb[:, c, n0:n1],
                            start=(ci == 0),
                            stop=(ci == 3),
                        )
                ot = outp.tile([128, EMB], F32, tag="out")
                if qc8 % 2 == 0:
                    nc.vector.tensor_copy(ot[:], pso[:])
                else:
                    nc.scalar.copy(ot[:], pso[:])
                nc.sync.dma_start(out[qc8 * 128 : (qc8 + 1) * 128, :], ot[:])

    nc.compile()
    return nc


def build_in_maps(x, Wqkv, bqkv, Wproj, bproj):
    x = np.asarray(x, dtype=np.float32)
    Wqkv = np.asarray(Wqkv, dtype=np.float32)
    bqkv = np.asarray(bqkv, dtype=np.float32)
    Wproj = np.asarray(Wproj, dtype=np.float32)
    bproj = np.asarray(bproj, dtype=np.float32)

    s = 1.0 / math.sqrt(EMB)
    cols = np.arange(3 * EMB).reshape(HEADS, DH, 3)  # (h, d, qkv) col index map
    in_maps = []
    for c in range(N_CORES):
        b, g = divmod(c, 2)
        hsl = slice(g * HPC, (g + 1) * HPC)
        qcols = cols[hsl, :, 0].reshape(-1)
        kcols = cols[hsl, :, 1].reshape(-1)
        vcols = cols[hsl, :, 2]  # [HPC, DH]

        xT_a = np.zeros((KC * 128, N), np.float32)
        xT_a[:EMB] = x[b].T
        xT_a[EMB] = 1.0

        Wq_a = np.zeros((KC * 128, 384), np.float32)
        Wq_a[:EMB] = Wqkv[:, qcols]
        Wq_a[EMB] = bqkv[qcols]
        Wk_a = np.zeros((KC * 128, 384), np.float32)
        Wk_a[:EMB] = Wqkv[:, kcols]
        Wk_a[EMB] = bqkv[kcols]

        Wv_a = np.zeros((KC * 128, HPC * 65), np.float32)
        for j in range(HPC):
            Wv_a[:EMB, j * 65 : j * 65 + DH] = Wqkv[:, vcols[j]]
            Wv_a[EMB, j * 65 : j * 65 + DH] = bqkv[vcols[j]]
            Wv_a[EMB, j * 65 + DH] = 1.0

        Wp_a = np.zeros((4 * 128, EMB), np.float32)
        Wp_a[:384] = Wproj[g * 384 : (g + 1) * 384] * s
        if g == 0:
            Wp_a[384] = bproj

        in_maps.append({"xT": xT_a, "Wq": Wq_a, "Wk": Wk_a, "Wv": Wv_a, "Wp": Wp_a})
    return in_maps


_NC_CACHE = None


def _get_program():
    global _NC_CACHE
    if _NC_CACHE is None:
        _NC_CACHE = build_program()
    return _NC_CACHE


def kernel(x, Wqkv, bqkv, Wproj, bproj, **_kwargs):
    nc = _get_program()
    in_maps = build_in_maps(x, Wqkv, bqkv, Wproj, bproj)
    res = run_bass_kernel_spmd(nc, in_maps, list(range(N_CORES))).results
    out = np.empty((B, N, EMB), np.float32)
    for b in range(B):
        out[b] = res[2 * b]["out"] + res[2 * b + 1]["out"]
    return out
